# revision 1
# baseline (speedup 1.0000x reference)
"""Trainium2 Bass kernel for DeepGraphConv survival model (GNN message passing).

Model (see reference):
  h  = relu(x @ W_fc + b_fc)                      [N, H]
  h1 = relu(GIN(h;  W1a,b1a,W1b,b1b))             GIN: relu(z@Wa+ba)@Wb+bb, z = h + seg_sum(h[src], dst)
  h2 = relu(GIN(h1; W2a,b2a,W2b,b2b))
  hp = [h|h1|h2]                                  [N, 3H]
  A  = (tanh(hp@Wa+ba) * sigmoid(hp@Wb+bb)) @ Wc + bc
  w  = segment_softmax(A, batch)                  per-graph
  pooled = seg_sum(hp * w, batch)                 [G, 3H]
  out = relu(pooled@Wr+br) @ Wk + bk              [G]

Distribution: nodes sharded contiguously across 8 cores (6250 each); incident
edges partitioned by dst owner; MLP weights replicated; per-layer bf16 node
feature tables AllGather'd; message gather via SWDGE dma_gather; segment-sum
via one-hot matmuls into PSUM; one [G, 3H+1] AllReduce for attention pooling.

On-device layout is feature-major (features on partitions, nodes on the free
axis) so every MLP matmul uses the natural [K, M] weight layout with no
activations transposes; x is transposed on entry via PE transposes.
"""

import sys

sys.path.insert(0, "/opt/trn_rl_repo")

import os

import numpy as np
import ml_dtypes

BF16 = ml_dtypes.bfloat16

# ---------------------------------------------------------------- problem cfg
FULL_CFG = dict(N=50000, E=800000, G=8, IN_DIM=1792, C=8)
H = 128
H3 = 3 * H
BLK = 128
MAX_GATHER_IDXS = 1024  # per dma_gather; SWDGE ring holds dynamic_dma_scratch_size//16 descs
MLP_W = 512  # node-tile width for MLP / attention matmuls


def _derived(cfg):
    N, C = cfg["N"], cfg["C"]
    assert N % C == 0
    S = N // C  # nodes per core
    NBLK = -(-S // BLK)  # dst blocks per core
    SP = NBLK * BLK  # padded node count per core
    lo_cap = cfg.get("LO_CAP", 32768)
    LO = min(lo_cap, N)  # rows addressable with int16 from base 0
    HI_BASE = max(N - lo_cap, 0)  # second gather base
    KC = cfg["IN_DIM"] // H
    assert cfg["IN_DIM"] % H == 0
    # groups of <=8 blocks (1024 dst columns -> 2 PSUM banks)
    groups = []
    b = 0
    while b < NBLK:
        nb = min(8, NBLK - b)
        groups.append((b, nb))
        b += nb
    return S, NBLK, SP, LO, HI_BASE, KC, groups


# ---------------------------------------------------------------- host plan


class Plan:
    """Static (uniform across cores) schedule + per-core data arrays."""

    pass


def make_plan(edge_index, batch, cfg):
    N, E, G, C = cfg["N"], cfg["E"], cfg["G"], cfg["C"]
    S, NBLK, SP, LO, HI_BASE, KC, groups = _derived(cfg)

    src = np.asarray(edge_index[0], dtype=np.int64)
    dst = np.asarray(edge_index[1], dtype=np.int64)
    batch = np.asarray(batch, dtype=np.int64)

    # per (core, block, pass) edge lists
    core = dst // S
    dloc = dst - core * S
    blk = dloc // BLK
    hi = (src >= LO).astype(np.int64)

    # counts[c, b, p]
    counts = np.zeros((C, NBLK, 2), dtype=np.int64)
    np.add.at(counts, (core, blk, hi), 1)

    # tiles per (block, pass): uniform across cores
    T = -(-counts.max(axis=0) // BLK)  # [NBLK, 2] ceil
    # group edges: order by (core, pass, block); within arbitrary
    order = np.lexsort((dst, hi, core))
    src_o, dst_o, core_o, blk_o, hi_o, dloc_o = (
        src[order], dst[order], core[order], blk[order], hi[order], dloc[order])

    # slice boundaries per (c, p, b) in the sorted order
    # key = ((c * 2 + p) * NBLK + b)
    key = (core_o * 2 + hi_o) * NBLK + blk_o
    # edges are sorted by (c, p, dst) => also by (c, p, b). key is nondecreasing.
    starts = np.searchsorted(key, np.arange(C * 2 * NBLK))
    ends = np.searchsorted(key, np.arange(C * 2 * NBLK) + 1)

    # ---- uniform tile schedule ------------------------------------------
    # per group: pass 0 blocks in order, then pass 1 blocks. chunks capped at
    # MAX_GATHER_IDXS, never crossing (g, p) boundaries, cut at tile bounds.
    chunks = []  # list of dict(g, p, off(idx cols), n_idx, tiles=[(b, first, last)])
    tile_seq = []  # global tile order: (chunk_id, slot_in_chunk, b, first, last)
    # which (b) has any tiles at all
    has_tiles = (T.sum(axis=1) > 0)

    # PSUM start=True zero-fills the whole 2KB zero region (one bank = 4
    # 128-col f32 windows), so start/stop must be at BANK granularity:
    # start on the first matmul touching a bank, stop on the last.
    bank_of = {}
    for g, (b0, nb) in enumerate(groups):
        for b in range(b0, b0 + nb):
            bank_of[b] = (g, (b - b0) // 4)
    first_seen = set()
    last_tile_of_bank = {}
    for g, (b0, nb) in enumerate(groups):
        for p in (0, 1):
            for b in range(b0, b0 + nb):
                if T[b, p] > 0:
                    last_tile_of_bank[bank_of[b]] = (p, b, T[b, p] - 1)

    for g, (b0, nb) in enumerate(groups):
        for p in (0, 1):
            cur = None
            for b in range(b0, b0 + nb):
                for i in range(T[b, p]):
                    if cur is None or cur["n_idx"] >= MAX_GATHER_IDXS:
                        cur = dict(g=g, p=p, off=0, n_idx=0, tiles=[])
                        chunks.append(cur)
                    bk = bank_of[b]
                    first = bk not in first_seen
                    first_seen.add(bk)
                    last = last_tile_of_bank.get(bk) == (p, b, i)
                    slot = cur["n_idx"] // BLK
                    cur["tiles"].append((b, first, last))
                    tile_seq.append((len(chunks) - 1, slot, b, first, last))
                    cur["n_idx"] += BLK
    # assign chunk offsets sequentially
    off = 0
    for ch in chunks:
        ch["off"] = off
        off += ch["n_idx"] // 16
    W_IDX = max(off, 1)
    TT = max(len(tile_seq), 1)

    st = Plan()
    st.cfg = dict(cfg)
    st.S, st.NBLK, st.SP, st.LO, st.HI_BASE, st.KC, st.groups = (
        S, NBLK, SP, LO, HI_BASE, KC, groups)
    st.T = T
    st.chunks = chunks
    st.tile_seq = tile_seq
    st.W_IDX = W_IDX
    st.TT = TT
    st.empty_blocks = [b for b in range(NBLK) if not has_tiles[b]]
    st.max_chunk_tiles = max((ch["n_idx"] // BLK for ch in chunks), default=1)

    # ---- per-core data ---------------------------------------------------
    st.eidx = np.zeros((C, 128, W_IDX), dtype=np.int16)
    st.dstloc = np.full((C, 128, TT), -1.0, dtype=np.float32)
    st.g1hot = np.zeros((C, 128, NBLK, G), dtype=BF16)
    for c in range(C):
        # graph one-hot over padded local nodes
        gids = batch[c * S:(c + 1) * S]  # [S]
        onehot = np.zeros((SP, G), dtype=BF16)
        onehot[np.arange(S), gids] = 1
        st.g1hot[c] = onehot.reshape(NBLK, BLK, G).transpose(1, 0, 2)

        idx_flat = np.zeros((W_IDX * 16,), dtype=np.int64)
        tglob = 0
        bpos = {}  # (p, b) -> edges consumed so far (blocks may span chunks)
        for ch in chunks:
            p = ch["p"]
            base = ch["off"] * 16
            pos = 0
            for (b, _f, _l) in ch["tiles"]:
                k = (c * 2 + p) * NBLK + b
                e0, e1 = starts[k], ends[k]
                done = bpos.get((p, b), 0)
                n_here = min(128, max(0, (e1 - e0) - done))
                tile_idx = np.zeros((128,), dtype=np.int64)
                tile_dl = np.full((128,), -1.0, dtype=np.float32)
                if n_here > 0:
                    sl = slice(e0 + done, e0 + done + n_here)
                    s_part = src_o[sl]
                    tile_idx[:n_here] = np.where(s_part < LO, s_part, s_part - HI_BASE)
                    tile_dl[:n_here] = dloc_o[sl] - b * BLK
                bpos[(p, b)] = done + n_here
                idx_flat[base + pos: base + pos + 128] = tile_idx
                st.dstloc[c, :, tglob] = tile_dl
                pos += 128
                tglob += 1
        assert tglob == len(tile_seq)
        wrapped = idx_flat.reshape(W_IDX, 16).T.astype(np.int16)  # [16, W]
        st.eidx[c] = np.tile(wrapped, (8, 1))

    # sanity: every real edge got placed exactly once
    n_placed = int((st.dstloc >= 0).sum())
    assert n_placed == E, (n_placed, E)
    return st


# ---------------------------------------------------------------- weights


def prep_weights(inp, cfg):
    """Host-side packing of (replicated) weights into SBUF-layout arrays."""
    KC = cfg["IN_DIM"] // H

    def f32(a):
        return np.ascontiguousarray(np.asarray(a, dtype=np.float32))

    def bf(a):
        return np.ascontiguousarray(np.asarray(a).astype(BF16))

    w = {}
    # fc: lhsT chunks [128, KC, H]
    w["wfc"] = bf(f32(inp["W_fc"]).reshape(KC, H, H).transpose(1, 0, 2))
    w["bfc"] = f32(inp["b_fc"]).reshape(H, 1)
    for nm in ("1a", "1b", "2a", "2b"):
        w["w" + nm] = bf(inp["W" + nm])  # [H, H]
        w["b" + nm] = f32(inp["b" + nm]).reshape(H, 1)
    # attention: [128, kc, mc, H]
    w["wa"] = bf(f32(inp["Wa"]).reshape(3, H, 3, H).transpose(1, 0, 2, 3))
    w["wb"] = bf(f32(inp["Wb"]).reshape(3, H, 3, H).transpose(1, 0, 2, 3))
    w["ba"] = f32(inp["ba"]).reshape(3, H).T.copy()  # [128, 3]
    w["bb"] = f32(inp["bb"]).reshape(3, H).T.copy()
    # Wc replicated across 128 M columns: [128, kc, 128]
    wc = f32(inp["Wc"]).reshape(3, H)  # [kc, 128(p)]
    w["wcr"] = bf(np.repeat(wc.transpose(1, 0)[:, :, None], H, axis=2))  # [128,3,128]
    w["bc"] = float(np.asarray(inp["bc"]).reshape(-1)[0])
    w["bcv"] = np.full((128, 1), w["bc"], dtype=np.float32)
    # final (fp32)
    w["wr"] = f32(inp["Wr"]).reshape(3, H, 3, H).transpose(1, 0, 2, 3).copy()
    w["br"] = f32(inp["br"]).reshape(3, H).T.copy()  # [128, 3]
    w["wk"] = f32(inp["Wk"]).reshape(3, H).T.copy()  # [128, 3]
    w["bk"] = float(np.asarray(inp["bk"]).reshape(-1)[0])
    w["iota"] = np.tile(np.arange(128, dtype=np.float32), (128, 1))
    ident = np.eye(128, dtype=np.float32)
    w["ident_f"] = ident
    w["ident_b"] = ident.astype(BF16)
    return w


# ---------------------------------------------------------------- bass build


def build_nc(st, reps=1):
    import concourse.bacc as bacc
    import concourse.tile as tile
    from concourse import library_config, mybir

    dt = mybir.dt
    AF = mybir.ActivationFunctionType
    OP = mybir.AluOpType
    cfg = st.cfg
    N, G, IN_DIM, C = cfg["N"], cfg["G"], cfg["IN_DIM"], cfg["C"]
    S, NBLK, SP, LO, HI_BASE, KC, groups = (
        st.S, st.NBLK, st.SP, st.LO, st.HI_BASE, st.KC, st.groups)

    nc = bacc.Bacc(None, target_bir_lowering=False, num_devices=C,
                   dynamic_dma_scratch_size=32768)

    ein = lambda nm, shp, d: nc.dram_tensor(nm, shp, d, kind="ExternalInput")
    xs = ein("xs", [SP, IN_DIM], dt.float32)
    eidx = ein("eidx", [128, st.W_IDX], dt.int16)
    dstloc = ein("dstloc", [128, st.TT], dt.float32)
    g1hot = ein("g1hot", [128, NBLK, G], dt.bfloat16)
    wfc = ein("wfc", [128, KC, H], dt.bfloat16)
    bfc = ein("bfc", [128, 1], dt.float32)
    wgin = {nm: ein("w" + nm, [H, H], dt.bfloat16) for nm in ("1a", "1b", "2a", "2b")}
    bgin = {nm: ein("b" + nm, [128, 1], dt.float32) for nm in ("1a", "1b", "2a", "2b")}
    wa = ein("wa", [128, 3, 3, H], dt.bfloat16)
    wb = ein("wb", [128, 3, 3, H], dt.bfloat16)
    ba = ein("ba", [128, 3], dt.float32)
    bb = ein("bb", [128, 3], dt.float32)
    wcr = ein("wcr", [128, 3, H], dt.bfloat16)
    bcv = ein("bcv", [128, 1], dt.float32)
    wr = ein("wr", [128, 3, 3, H], dt.float32)
    br = ein("br", [128, 3], dt.float32)
    wk = ein("wk", [128, 3], dt.float32)
    iota_i = ein("iota", [128, 128], dt.float32)
    idf_i = ein("ident_f", [128, 128], dt.float32)
    idb_i = ein("ident_b", [128, 128], dt.bfloat16)
    out_t = nc.dram_tensor("out", [1, G], dt.float32, kind="ExternalOutput")
    bc_const = st.weights["bc"]
    bk_const = st.weights["bk"]

    rg = [list(range(C))]
    stage = int(os.environ.get("DEBUG_STAGE", "9"))

    with tile.TileContext(nc, num_cores=C) as tc:
        nc.gpsimd.load_library(library_config.mlp)
        with (
            tc.tile_pool(name="dram", bufs=1, space="DRAM") as dram,
            tc.tile_pool(name="consts", bufs=1) as consts,
            tc.tile_pool(name="persist", bufs=1) as persist,
        ):
            # internal DRAM
            h_own = dram.tile([S, H], dt.bfloat16, tag="h_own")
            h_full = dram.tile([N, H], dt.bfloat16, tag="h_full")
            h1_own = dram.tile([S, H], dt.bfloat16, tag="h1_own")
            h1_full = dram.tile([N, H], dt.bfloat16, tag="h1_full")
            cc_in = dram.tile([G, H3 + 1], dt.float32, tag="cc_in")
            cc_out = dram.tile([G, H3 + 1], dt.float32, tag="cc_out")

            # constants into SBUF
            def load_const(ap, shape, d):
                t = consts.tile(shape, d, tag="c_" + ap.name)
                nc.sync.dma_start(out=t[:], in_=ap[:])
                return t

            wfc_s = load_const(wfc, [128, KC, H], dt.bfloat16)
            bfc_s = load_const(bfc, [128, 1], dt.float32)
            wgin_s = {k: load_const(v, [H, H], dt.bfloat16) for k, v in wgin.items()}
            bgin_s = {k: load_const(v, [128, 1], dt.float32) for k, v in bgin.items()}
            wa_s = load_const(wa, [128, 3, 3, H], dt.bfloat16)
            wb_s = load_const(wb, [128, 3, 3, H], dt.bfloat16)
            ba_s = load_const(ba, [128, 3], dt.float32)
            bb_s = load_const(bb, [128, 3], dt.float32)
            wcr_s = load_const(wcr, [128, 3, H], dt.bfloat16)
            bcv_s = load_const(bcv, [128, 1], dt.float32)
            wr_s = load_const(wr, [128, 3, 3, H], dt.float32)
            br_s = load_const(br, [128, 3], dt.float32)
            wk_s = load_const(wk, [128, 3], dt.float32)
            iota = load_const(iota_i, [128, 128], dt.float32)
            idf = load_const(idf_i, [128, 128], dt.float32)
            idb = load_const(idb_i, [128, 128], dt.bfloat16)
            eidx_s = load_const(eidx, [128, st.W_IDX], dt.int16)
            dstloc_s = load_const(dstloc, [128, st.TT], dt.float32)
            g1hot_s = load_const(g1hot, [128, NBLK, G], dt.bfloat16)

            # persistent activations (feature-major)
            hT = persist.tile([128, SP], dt.bfloat16, tag="hT")
            h1T = persist.tile([128, SP], dt.bfloat16, tag="h1T")
            h2T = persist.tile([128, SP], dt.bfloat16, tag="h2T")
            e_rep = persist.tile([128, SP], dt.float32, tag="e_rep")

            # node tiles of width MLP_W (last one possibly short)
            def ntiles(width_total):
                tl = []
                o = 0
                while o < width_total:
                    w = min(MLP_W, width_total - o)
                    tl.append((o, w))
                    o += w
                return tl

            for _rep in range(reps):
              # ---------------- phase 1: fc ----------------
              with (
                  tc.tile_pool(name="xf", bufs=3) as xfp,
                  tc.tile_pool(name="xb", bufs=2) as xbp,
                  tc.tile_pool(name="xT", bufs=2) as xTp,
                  tc.tile_pool(name="tp_ps", bufs=4, space="PSUM") as tpps,
                  tc.tile_pool(name="h_ps", bufs=2, space="PSUM") as hps,
              ):
                  for (o, wdt) in ntiles(SP):
                      nt = -(-wdt // 128)
                      xTt = xTp.tile([128, KC, MLP_W], dt.bfloat16, tag="xT")
                      for j in range(nt):
                          xf_t = xfp.tile([128, IN_DIM], dt.float32, tag="xf")
                          nc.sync.dma_start(
                              out=xf_t[:], in_=xs[o + j * 128: o + (j + 1) * 128, :])
                          xb_t = xbp.tile([128, IN_DIM], dt.bfloat16, tag="xb")
                          nc.vector.tensor_copy(out=xb_t[:], in_=xf_t[:])
                          for kc in range(KC):
                              ps = tpps.tile([128, 128], dt.bfloat16, tag="tp")
                              nc.tensor.transpose(
                                  ps[:], xb_t[:, kc * 128:(kc + 1) * 128], idb[:])
                              nc.scalar.copy(
                                  out=xTt[:, kc, j * 128:(j + 1) * 128], in_=ps[:])
                      hp = hps.tile([128, MLP_W], dt.float32, tag="hps")
                      for kc in range(KC):
                          nc.tensor.matmul(
                              hp[:, :wdt], lhsT=wfc_s[:, kc, :], rhs=xTt[:, kc, :wdt],
                              start=(kc == 0), stop=(kc == KC - 1))
                      nc.scalar.activation(
                          hT[:, o:o + wdt], hp[:, :wdt], AF.Relu, bias=bfc_s[:])

              # write node-major h table + AllGather
              def write_table(srcT, own, full):
                  with (
                      tc.tile_pool(name="wt_ps", bufs=4, space="PSUM") as wtps,
                      tc.tile_pool(name="wt_sb", bufs=4) as wtsb,
                  ):
                      for tb in range(NBLK):
                          o = tb * 128
                          wdt = min(128, S - o)
                          ps = wtps.tile([128, 128], dt.bfloat16, tag="wt")
                          nc.tensor.transpose(
                              ps[:wdt, :], srcT[:, o:o + wdt], idb[:])
                          nm = wtsb.tile([128, 128], dt.bfloat16, tag="wtsb")
                          nc.scalar.copy(out=nm[:wdt, :], in_=ps[:wdt, :])
                          nc.sync.dma_start(out=own[o:o + wdt, :], in_=nm[:wdt, :])
                  if os.environ.get("DEBUG_NO_CC"):
                      nc.sync.dma_start(out=full[0:S, :], in_=own[:])
                  else:
                      nc.gpsimd.collective_compute(
                          "AllGather", mybir.AluOpType.bypass, replica_groups=rg,
                          ins=[own[:].opt()], outs=[full[:].opt()])

              if stage >= 2:
                  write_table(hT, h_own, h_full)

              # ---------------- GIN layers ----------------
              def gin_layer(tabT, full_tab, outT, wA, bA, wB, bB, write_out):
                  with (
                      tc.tile_pool(name="msgs", bufs=2) as msgs,
                      tc.tile_pool(name="oh", bufs=4) as ohp,
                      tc.tile_pool(name="agg_ps", bufs=2, space="PSUM") as aggp,
                      tc.tile_pool(name="mlp_ps", bufs=2, space="PSUM") as mlpp,
                      tc.tile_pool(name="zb", bufs=3) as zbp,
                  ):
                      tglob = 0
                      ci = 0
                      for g, (b0, nb) in enumerate(groups):
                          agg = aggp.tile([128, 8 * 128], dt.float32, tag="agg")
                          # empty blocks -> zero their window
                          for b in range(b0, b0 + nb):
                              if b in st.empty_blocks:
                                  nc.vector.memset(
                                      agg[:, (b - b0) * 128:(b - b0 + 1) * 128], 0.0)
                          while ci < len(st.chunks) and st.chunks[ci]["g"] == g:
                              ch = st.chunks[ci]
                              n_idx = ch["n_idx"]
                              ntl = n_idx // 128
                              m = msgs.tile(
                                  [128, st.max_chunk_tiles, H], dt.bfloat16, tag="m")
                              base = 0 if ch["p"] == 0 else HI_BASE
                              nrows = LO if ch["p"] == 0 else N - HI_BASE
                              nc.gpsimd.dma_gather(
                                  m[:, :ntl, :],
                                  full_tab[base:base + nrows, :],
                                  eidx_s[:, ch["off"]: ch["off"] + n_idx // 16],
                                  n_idx, n_idx, H, elem_step=H)
                              for slot, (b, first, last) in enumerate(ch["tiles"]):
                                  oh = ohp.tile([128, 128], dt.bfloat16, tag="oh")
                                  nc.vector.tensor_scalar(
                                      out=oh[:], in0=iota[:],
                                      scalar1=dstloc_s[:, tglob:tglob + 1],
                                      scalar2=None, op0=OP.is_equal)
                                  w0 = (b - b0) * 128
                                  nc.tensor.matmul(
                                      agg[:, w0:w0 + 128], lhsT=m[:, slot, :],
                                      rhs=oh[:], start=first, stop=last,
                                      skip_group_check=True)
                                  tglob += 1
                              ci += 1
                          # MLP over this group's node columns
                          go = b0 * 128
                          gw = nb * 128
                          for (o, wdt) in ntiles(gw):
                              z = zbp.tile([128, MLP_W], dt.bfloat16, tag="z")
                              nc.vector.tensor_tensor(
                                  out=z[:, :wdt], in0=agg[:, o:o + wdt],
                                  in1=tabT[:, go + o:go + o + wdt], op=OP.add)
                              p1 = mlpp.tile([128, MLP_W], dt.float32, tag="mlp")
                              nc.tensor.matmul(
                                  p1[:, :wdt], lhsT=wA[:], rhs=z[:, :wdt],
                                  start=True, stop=True)
                              y1 = zbp.tile([128, MLP_W], dt.bfloat16, tag="y1")
                              nc.scalar.activation(
                                  y1[:, :wdt], p1[:, :wdt], AF.Relu, bias=bA[:])
                              p2 = mlpp.tile([128, MLP_W], dt.float32, tag="mlp")
                              nc.tensor.matmul(
                                  p2[:, :wdt], lhsT=wB[:], rhs=y1[:, :wdt],
                                  start=True, stop=True)
                              nc.scalar.activation(
                                  outT[:, go + o:go + o + wdt], p2[:, :wdt],
                                  AF.Relu, bias=bB[:])
                      assert tglob == st.TT, (tglob, st.TT)
                  if write_out is not None:
                      write_table(outT, write_out[0], write_out[1])

              if stage >= 3:
                  gin_layer(hT, h_full, h1T, wgin_s["1a"], bgin_s["1a"],
                            wgin_s["1b"], bgin_s["1b"],
                            (h1_own, h1_full) if stage >= 4 else None)
              if stage >= 4:
                  gin_layer(h1T, h1_full, h2T, wgin_s["2a"], bgin_s["2a"],
                            wgin_s["2b"], bgin_s["2b"], None)

              # ---------------- attention + pooling ----------------
              if stage < 5:
                  continue
              hp_chunks = [hT, h1T, h2T]
              with (
                  tc.tile_pool(name="at_ps", bufs=3, space="PSUM") as atps,
                  tc.tile_pool(name="at_sb", bufs=4) as atsb,
              ):
                  for (o, wdt) in ntiles(SP):
                      gated = atsb.tile([128, 3, MLP_W], dt.bfloat16, tag="gated")
                      for mc in range(3):
                          pA = atps.tile([128, MLP_W], dt.float32, tag="at")
                          for kc in range(3):
                              nc.tensor.matmul(
                                  pA[:, :wdt], lhsT=wa_s[:, kc, mc, :],
                                  rhs=hp_chunks[kc][:, o:o + wdt],
                                  start=(kc == 0), stop=(kc == 2))
                          tanh_t = atsb.tile([128, MLP_W], dt.bfloat16, tag="tanh")
                          nc.scalar.activation(
                              tanh_t[:, :wdt], pA[:, :wdt], AF.Tanh,
                              bias=ba_s[:, mc:mc + 1])
                          pB = atps.tile([128, MLP_W], dt.float32, tag="at")
                          for kc in range(3):
                              nc.tensor.matmul(
                                  pB[:, :wdt], lhsT=wb_s[:, kc, mc, :],
                                  rhs=hp_chunks[kc][:, o:o + wdt],
                                  start=(kc == 0), stop=(kc == 2))
                          sig_t = atsb.tile([128, MLP_W], dt.bfloat16, tag="sig")
                          nc.scalar.activation(
                              sig_t[:, :wdt], pB[:, :wdt], AF.Sigmoid,
                              bias=bb_s[:, mc:mc + 1])
                          nc.vector.tensor_tensor(
                              out=gated[:, mc, :wdt], in0=tanh_t[:, :wdt],
                              in1=sig_t[:, :wdt], op=OP.mult)
                      pS = atps.tile([128, MLP_W], dt.float32, tag="at")
                      for kc in range(3):
                          nc.tensor.matmul(
                              pS[:, :wdt], lhsT=wcr_s[:, kc, :],
                              rhs=gated[:, kc, :wdt],
                              start=(kc == 0), stop=(kc == 2))
                      nc.scalar.activation(
                          e_rep[:, o:o + wdt], pS[:, :wdt], AF.Exp, bias=bcv_s[:])
                      # weight hp by e in-place (bf16)
                      for mc in range(3):
                          nc.vector.tensor_tensor(
                              out=hp_chunks[mc][:, o:o + wdt],
                              in0=hp_chunks[mc][:, o:o + wdt],
                              in1=e_rep[:, o:o + wdt], op=OP.mult)

              # pooled[G, 385] = sum_n g1hot[n,:]^T (hpw | e)
              with (
                  tc.tile_pool(name="pool_ps", bufs=1, space="PSUM") as plps,
                  tc.tile_pool(name="tp2_ps", bufs=3, space="PSUM") as tp2,
                  tc.tile_pool(name="tpe_ps", bufs=2, space="PSUM") as tpe,
                  tc.tile_pool(name="rhs_sb", bufs=3) as rhsp,
              ):
                  pooled_ps = plps.tile([G, H3 + 1], dt.float32, tag="pool")
                  for tb in range(NBLK):
                      o = tb * 128
                      rhs_t = rhsp.tile([128, H3 + 8], dt.bfloat16, tag="rhs")
                      for mc in range(3):
                          ps = tp2.tile([128, 128], dt.bfloat16, tag="tp2")
                          nc.tensor.transpose(
                              ps[:], hp_chunks[mc][:, o:o + 128], idb[:])
                          nc.scalar.copy(
                              out=rhs_t[:, mc * 128:(mc + 1) * 128], in_=ps[:])
                      pe = tpe.tile([128, 128], dt.float32, tag="tpe")
                      nc.tensor.transpose(pe[:], e_rep[:, o:o + 128], idf[:])
                      nc.scalar.copy(out=rhs_t[:, H3:H3 + 1], in_=pe[:, :1])
                      nc.tensor.matmul(
                          pooled_ps[:], lhsT=g1hot_s[:, tb, :],
                          rhs=rhs_t[:, :H3 + 1],
                          start=(tb == 0), stop=(tb == NBLK - 1))
                  pooled_sb = rhsp.tile([G, H3 + 1], dt.float32, tag="pooled_sb")
                  nc.vector.tensor_copy(out=pooled_sb[:], in_=pooled_ps[:])
                  nc.sync.dma_start(out=cc_in[:], in_=pooled_sb[:])

              if os.environ.get("DEBUG_NO_CC"):
                  nc.sync.dma_start(out=cc_out[:], in_=cc_in[:])
              else:
                  nc.gpsimd.collective_compute(
                      "AllReduce", mybir.AluOpType.add, replica_groups=rg,
                      ins=[cc_in[:].opt()], outs=[cc_out[:].opt()])

              # ---------------- final MLP (fp32) ----------------
              with (
                  tc.tile_pool(name="fin_sb", bufs=1) as fsb,
                  tc.tile_pool(name="fin_ps", bufs=2, space="PSUM") as fps,
              ):
                  pl = fsb.tile([G, H3 + 1], dt.float32, tag="pl")
                  nc.sync.dma_start(out=pl[:], in_=cc_out[:])
                  rd = fsb.tile([G, 1], dt.float32, tag="rd")
                  nc.vector.reciprocal(rd[:], pl[:, H3:H3 + 1])
                  nc.vector.tensor_scalar(
                      out=pl[:, :H3], in0=pl[:, :H3], scalar1=rd[:],
                      scalar2=None, op0=mybir.AluOpType.mult)
                  plT = fsb.tile([128, 3, G], dt.float32, tag="plT")
                  for kc in range(3):
                      ps = fps.tile([128, G], dt.float32, tag="fpt")
                      nc.tensor.transpose(
                          ps[:], pl[:G, kc * 128:(kc + 1) * 128], idf[:G, :G])
                      nc.vector.tensor_copy(out=plT[:, kc, :], in_=ps[:])
                  rT = fsb.tile([128, 3, G], dt.float32, tag="rT")
                  for mc in range(3):
                      ps = fps.tile([128, G], dt.float32, tag="fpr")
                      for kc in range(3):
                          nc.tensor.matmul(
                              ps[:], lhsT=wr_s[:, kc, mc, :], rhs=plT[:, kc, :],
                              start=(kc == 0), stop=(kc == 2))
                      nc.scalar.activation(
                          rT[:, mc, :], ps[:], AF.Relu, bias=br_s[:, mc:mc + 1])
                  po = fps.tile([1, G], dt.float32, tag="fpo")
                  for mc in range(3):
                      nc.tensor.matmul(
                          po[:], lhsT=wk_s[:, mc:mc + 1], rhs=rT[:, mc, :],
                          start=(mc == 0), stop=(mc == 2))
                  ob = fsb.tile([1, G], dt.float32, tag="ob")
                  nc.scalar.activation(ob[:], po[:], AF.Copy, bias=bk_const)
                  nc.sync.dma_start(out=out_t[:], in_=ob[:])

            if stage < 5:
                with tc.tile_pool(name="stub", bufs=1) as stub:
                    sb = stub.tile([1, G], dt.float32, tag="stub")
                    nc.vector.tensor_copy(out=sb[:], in_=iota[:1, :G])
                    nc.sync.dma_start(out=out_t[:], in_=sb[:])

    nc.finalize()
    return nc


# ---------------------------------------------------------------- entry


def _make_in_maps(inputs, st, w):
    cfg = st.cfg
    C, S, SP, IN_DIM = cfg["C"], st.S, st.SP, cfg["IN_DIM"]
    x = np.asarray(inputs["x"], dtype=np.float32)
    in_maps = []
    for c in range(C):
        xs = np.zeros((SP, IN_DIM), dtype=np.float32)
        xs[:S] = x[c * S:(c + 1) * S]
        m = dict(
            xs=xs,
            eidx=st.eidx[c],
            dstloc=st.dstloc[c],
            g1hot=st.g1hot[c],
            wfc=w["wfc"], bfc=w["bfc"],
            w1a=w["w1a"], b1a=w["b1a"], w1b=w["w1b"], b1b=w["b1b"],
            w2a=w["w2a"], b2a=w["b2a"], w2b=w["w2b"], b2b=w["b2b"],
            wa=w["wa"], wb=w["wb"], ba=w["ba"], bb=w["bb"], wcr=w["wcr"],
            wr=w["wr"], br=w["br"], wk=w["wk"], bcv=w["bcv"],
            iota=w["iota"], ident_f=w["ident_f"], ident_b=w["ident_b"],
        )
        in_maps.append(m)
    return in_maps


_LAST = {}


def _run(inputs, cfg, trace=False):
    from concourse.bass_utils import run_bass_kernel_spmd

    st = make_plan(inputs["edge_index"], inputs["batch"], cfg)
    w = prep_weights(inputs, cfg)
    st.weights = w
    nc = build_nc(st)
    in_maps = _make_in_maps(inputs, st, w)
    res = run_bass_kernel_spmd(
        nc, in_maps, core_ids=list(range(cfg["C"])), trace=trace)
    _LAST["result"] = res
    _LAST["nc"] = nc
    _LAST["st"] = st
    return np.asarray(res.results[0]["out"], dtype=np.float32).reshape(cfg["G"])


def kernel(**inputs) -> np.ndarray:
    return _run(inputs, FULL_CFG, trace=False)

